# revision 1
# baseline (speedup 1.0000x reference)
"""Transformer encoder layer (B=2, S=2048, D=1024, H=16) on 8 TRN2 NeuronCores.

Sharding: token-parallel (512 tokens/core), with per-batch AllGather of K/V
(replica groups [[0,1,2,3],[4,5,6,7]] — cores 0-3 own batch 0's tokens).

Per core (all activations d-major, [d_partition, token_free]):
  LN1 -> QKV for all 16 heads -> (2 waves) AllGather K,V within group ->
  attention for the core's 512 queries over the batch's 2048 keys ->
  W_o + residual -> LN2 -> FFN + residual -> LN3 -> output slice.

Softmax runs over the partition axis: scores are built transposed
(S^T[t, s] = k_t . q_s), exp on ACT, and the denominator comes for free as
a ones-column appended to V in the ctx matmul (out[64] = sum_t E[t, s]).
All matmuls use float32r (full PE rate for moving dim >= 256, ~1.5e-4 rel).
"""
import os
import sys

for _p in ("/opt/trn_rl_repo", "/root/.axon_site/_ro/trn_rl_repo"):
    if os.path.isdir(_p) and _p not in sys.path:
        sys.path.insert(0, _p)
        break

import numpy as np

B, S, D, H, DH = 2, 2048, 1024, 16, 64
P = 128          # partitions
TOK = 512        # tokens per core
DT = 8           # d tiles (D / P)
HP = 8           # head pairs
NC = 8
NSH = 4          # shards per replica group
EPS = 1e-5

TRACE = False    # set by test.py to get exec_time_ns
_BUILT = {}


def _build(reps=1, ag_groups=None, rep_attn=1, rep_tail=1, rep_head=1, rep_kv=1, rep_ln=1, exp_mode='act'):
    import concourse.mybir as mybir
    import concourse.tile as tile
    from concourse import bacc

    F32 = mybir.dt.float32
    F32R = mybir.dt.float32r
    BF16 = mybir.dt.bfloat16
    AF = mybir.ActivationFunctionType
    OP = mybir.AluOpType

    nc = bacc.Bacc(trn_type="TRN2", num_devices=NC, target_bir_lowering=False)

    # ---- I/O ----
    xT = nc.dram_tensor("xT", [P, DT, TOK], F32R, kind="ExternalInput")
    wq_in = nc.dram_tensor("wq", [8, P, DT, 128], F32R, kind="ExternalInput")
    wk_in = nc.dram_tensor("wk", [8, P, DT, 128], F32R, kind="ExternalInput")
    wv_in = nc.dram_tensor("wv", [2, DT, P, 512], F32R, kind="ExternalInput")
    wo_in = nc.dram_tensor("wo", [8, P, DT, 128], F32R, kind="ExternalInput")
    w1_in = nc.dram_tensor("w1", [8, P, DT, 128], F32R, kind="ExternalInput")
    w2_in = nc.dram_tensor("w2", [8, P, DT, 128], F32R, kind="ExternalInput")
    gb_in = nc.dram_tensor("gb", [P, 3, 2, DT], F32, kind="ExternalInput")
    outT = nc.dram_tensor("outT", [P, DT, TOK], F32, kind="ExternalOutput")

    with tile.TileContext(nc) as tc:
        with (
            tc.tile_pool(name="cst", bufs=1) as cst,
            tc.tile_pool(name="big", bufs=1) as big,
            tc.tile_pool(name="res", bufs=1) as res,
            tc.tile_pool(name="wts", bufs=6) as wts,
            tc.tile_pool(name="wvs", bufs=3) as wvs,
            tc.tile_pool(name="stg", bufs=2) as stg,
            tc.tile_pool(name="kvs", bufs=3) as kvs,
            tc.tile_pool(name="exps", bufs=4) as exps,
            tc.tile_pool(name="rows", bufs=1) as rows,
            tc.tile_pool(name="outs", bufs=2) as outs,
            tc.tile_pool(name="pmm", bufs=2, space="PSUM") as pmm,
            tc.tile_pool(name="pctx", bufs=1, space="PSUM") as pctx,
            tc.tile_pool(name="pst", bufs=1, space="PSUM") as pst,
            tc.tile_pool(name="dram", bufs=1, space="DRAM") as dram,
        ):
            # ---- constants ----
            ones_f = cst.tile([P, 1], F32)
            nc.vector.memset(ones_f, 1.0)
            ones_r = cst.tile([P, 1], F32R)
            nc.vector.tensor_copy(ones_r[:], ones_f[:])
            onesrow_f = cst.tile([1, P], F32)
            nc.vector.memset(onesrow_f, 1.0)
            ones_row = cst.tile([1, P], F32R)
            nc.vector.tensor_copy(ones_row[:], onesrow_f[:])
            eps_t = cst.tile([1, 1], F32)
            nc.vector.memset(eps_t, EPS)
            gb = cst.tile([P, 3, 2, DT], F32)
            nc.sync.dma_start(gb[:], gb_in[:])

            # ---- body (emitted `reps` times for benchmarking) ----
            for _rep in range(reps):
                # ---- load x^T ----
                xt = big.tile([P, DT, TOK], F32R, tag="xt")
                nc.sync.dma_start(xt[:], xT[:])

                def layernorm(src, out_tile, idx):
                    """src, out_tile: [P, DT, TOK]; idx: which ln params (0/1/2)."""
                    scr = big.tile([P, DT, TOK], F32R, tag="scratch")
                    nc.vector.tensor_mul(scr[:], src[:].bitcast(F32), src[:].bitcast(F32))
                    st0 = pst.tile([1, TOK], F32, tag="st0")
                    st1 = pst.tile([1, TOK], F32, tag="st1")
                    for dt in range(DT):
                        nc.tensor.matmul(st0[:], ones_r[:], src[:, dt, :],
                                         start=(dt == 0), stop=(dt == DT - 1))
                    for dt in range(DT):
                        nc.tensor.matmul(st1[:], ones_r[:], scr[:, dt, :],
                                         start=(dt == 0), stop=(dt == DT - 1))
                    mu = rows.tile([1, TOK], F32, tag="mu")
                    var = rows.tile([1, TOK], F32, tag="var")
                    msq = rows.tile([1, TOK], F32, tag="msq")
                    sd = rows.tile([1, TOK], F32, tag="sd")
                    row = rows.tile([1, 2 * TOK], F32R, tag="row")
                    nc.vector.tensor_scalar_mul(mu[:], st0[:], 1.0 / D)
                    nc.vector.tensor_scalar_mul(var[:], st1[:], 1.0 / D)
                    nc.vector.tensor_mul(msq[:], mu[:], mu[:])
                    nc.vector.tensor_tensor(var[:], var[:], msq[:], OP.subtract)
                    nc.scalar.activation(sd[:], var[:], AF.Sqrt, bias=eps_t[:], scale=1.0)
                    with nc.allow_low_precision(reason="f32r == f32 bits"):
                        nc.vector.reciprocal(row[:, 0:TOK], sd[:])
                    nc.vector.tensor_scalar_mul(msq[:], mu[:], -1.0)
                    nc.vector.tensor_mul(row[:, TOK:], msq[:], row[:, 0:TOK])
                    bcp = pmm.tile([P, 2, TOK], F32, tag="mm2")
                    nc.tensor.matmul(bcp[:, 0, :], ones_row[:], row[:, 0:TOK],
                                     start=True, stop=True)
                    nc.tensor.matmul(bcp[:, 1, :], ones_row[:], row[:, TOK:],
                                     start=True, stop=True)
                    rs_b = bcp[:, 0, None, :].to_broadcast((P, DT, TOK))
                    nb_b = bcp[:, 1, None, :].to_broadcast((P, DT, TOK))
                    nc.vector.tensor_mul(scr[:], src[:].bitcast(F32), rs_b)
                    nc.vector.tensor_tensor(scr[:], scr[:].bitcast(F32), nb_b, OP.add)
                    for dt in range(DT):
                        nc.scalar.activation(
                            out_tile[:, dt, :], scr[:, dt, :].bitcast(F32), AF.Identity,
                            scale=gb[:, idx, 0, dt:dt + 1], bias=gb[:, idx, 1, dt:dt + 1])

                # ---- LN1 ----
                ht = big.tile([P, DT, TOK], F32R, tag="act")
                for _ln in range(rep_ln):
                    layernorm(xt, ht, 0)

                # ---- K, V in two waves, each followed by its AllGather ----
                kv_out = []
                for _kv in range(rep_kv):
                  kv_out = []
                  for wave in range(2):
                    kv_in_w = dram.tile([P, NSH, 1032], BF16,
                                        name=f"kvin{_rep}_{_kv}_{wave}")
                    for j in range(4):
                        cb = wave * 4 + j
                        wt = wts.tile([P, DT, 128], F32R, tag="wt")
                        nc.sync.dma_start(wt[:], wk_in[cb])
                        ps2 = pmm.tile([P, 2, TOK], F32, tag="mm2",
                                       name=f"kps{wave}_{j}")
                        ps = ps2[:, 0, :]
                        for dt in range(DT):
                            nc.tensor.matmul(ps, wt[:, dt, :], ht[:, dt, :],
                                             start=(dt == 0), stop=(dt == DT - 1))
                        kst = stg.tile([P, TOK], BF16, tag="kstg")
                        nc.vector.tensor_copy(kst[:], ps[:])
                        nc.sync.dma_start(kv_in_w[:, j, 0:512], kst[:])
                    vst = stg.tile([P, 4, 8, 65], BF16, tag="vstg")
                    vp2 = [pmm.tile([P, 2, TOK], F32, tag="mm2",
                                    name=f"vp2{wave}_{i}") for i in range(2)]
                    vps = [vp2[i // 2][:, i % 2, :] for i in range(4)]
                    for dt in range(DT):
                        wvt = wvs.tile([P, 512], F32R, tag="wv")
                        nc.sync.dma_start(wvt[:], wv_in[wave, dt])
                        for tt in range(4):
                            nc.tensor.matmul(
                                vps[tt], ht[:, dt, tt * 128:(tt + 1) * 128], wvt[:],
                                start=(dt == 0), stop=(dt == DT - 1))
                    for tt in range(4):
                        nc.vector.tensor_copy(
                            vst[:, tt, :, 0:64],
                            vps[tt].rearrange("p (h w) -> p h w", w=64))
                    nc.vector.tensor_copy(
                        vst[:, :, :, 64:65],
                        ones_f[:, None, None, :].to_broadcast((P, 4, 8, 1)))
                    nc.sync.dma_start(
                        kv_in_w[:, :, 512:1032],
                        vst[:].rearrange("p t h w -> p t (h w)"))
                    kv_out_w = dram.tile([NSH, P, NSH, 1032], BF16,
                                         name=f"kvout{_rep}_{_kv}_{wave}")
                    nc.gpsimd.collective_compute(
                        "AllGather", mybir.AluOpType.bypass,
                        replica_groups=(ag_groups or [[0, 1, 2, 3], [4, 5, 6, 7]]),
                        ins=[kv_in_w.opt()], outs=[kv_out_w.opt()])
                    kv_out.append(kv_out_w)

                # ---- Q (scaled by 1/sqrt(DH)) ----
                qt = big.tile([P, HP, TOK], BF16, tag="qt")
                for cb in [c for _ in range(rep_head) for c in range(8)]:
                    wt = wts.tile([P, DT, 128], F32R, tag="wt")
                    nc.sync.dma_start(wt[:], wq_in[cb])
                    ps2 = pmm.tile([P, 2, TOK], F32, tag="mm2", name=f"qps{cb}")
                    ps = ps2[:, 0, :]
                    for dt in range(DT):
                        nc.tensor.matmul(ps, wt[:, dt, :], ht[:, dt, :],
                                         start=(dt == 0), stop=(dt == DT - 1))
                    nc.vector.tensor_scalar_mul(qt[:, cb, :], ps, 1.0 / np.sqrt(DH))

                # ---- attention ----
                ctx_sb = big.tile([P, HP, TOK], F32R, tag="ctx")
                for hp in [h for _ in range(rep_attn) for h in range(HP)]:
                    wave, hpl = hp // 4, hp % 4
                    ctxA = pctx.tile([P, TOK], F32, tag="ctxA")
                    ctxB = pctx.tile([P, TOK], F32, tag="ctxB")
                    pend = []        # (g, eA, eB, vtt) awaiting ctx matmuls
                    vtts = {}
                    for r in range(NSH):
                        ktt = kvs.tile([P, TOK], BF16, tag="ktt")
                        nc.sync.dma_start(ktt[:], kv_out[wave][r, :, hpl, 0:512])
                        vtt = kvs.tile([P, 4, 2, 65], BF16, tag="vtt")
                        c0 = 512 + (2 * hpl) * 65
                        nc.sync.dma_start(
                            vtt[:],
                            kv_out[wave][r, :, :, c0:c0 + 130]
                            .rearrange("p t (h w) -> p t h w", w=65))
                        for tt in range(4):
                            g = r * 4 + tt
                            s2 = pmm.tile([P, 2, TOK], F32, tag="mm2",
                                          name=f"s2_{hp}_{g}")
                            nc.tensor.matmul(s2[:, 0, :],
                                             ktt[0:64, tt * 128:(tt + 1) * 128],
                                             qt[0:64, hp, :], start=True, stop=True)
                            nc.tensor.matmul(s2[:, 1, :],
                                             ktt[64:128, tt * 128:(tt + 1) * 128],
                                             qt[64:128, hp, :], start=True, stop=True)
                            e2 = exps.tile([P, 2, TOK], BF16, tag="e",
                                           name=f"e2_{hp}_{g}")
                            nc.scalar.activation(e2[:], s2[:], AF.Exp)
                            pend.append((g, e2, vtt, tt))
                            if len(pend) > 3:
                                g0, e0, vt0, tt0 = pend.pop(0)
                                nc.tensor.matmul(ctxA[0:65, :], vt0[:, tt0, 0, :],
                                                 e0[:, 0, :], start=(g0 == 0),
                                                 stop=(g0 == 15))
                                nc.tensor.matmul(ctxB[0:65, :], vt0[:, tt0, 1, :],
                                                 e0[:, 1, :], start=(g0 == 0),
                                                 stop=(g0 == 15))
                    for g0, e0, vt0, tt0 in pend:
                        nc.tensor.matmul(ctxA[0:65, :], vt0[:, tt0, 0, :], e0[:, 0, :],
                                         start=(g0 == 0), stop=(g0 == 15))
                        nc.tensor.matmul(ctxB[0:65, :], vt0[:, tt0, 1, :], e0[:, 1, :],
                                         start=(g0 == 0), stop=(g0 == 15))
                    recA = rows.tile([1, TOK], F32R, tag="rec")
                    with nc.allow_low_precision(reason="f32r == f32 bits"):
                        nc.vector.reciprocal(recA[:], ctxA[64:65, :])
                    rbA = pst.tile([64, TOK], F32, tag="st0")
                    nc.tensor.matmul(rbA[:], ones_row[:, 0:64], recA[:],
                                     start=True, stop=True)
                    rbAs = rows.tile([64, TOK], F32, tag="rbs")
                    nc.vector.tensor_copy(rbAs[:], rbA[:])
                    nc.vector.tensor_mul(ctx_sb[0:64, hp, :], ctxA[0:64, :], rbAs[:])
                    recB = rows.tile([1, TOK], F32R, tag="rec")
                    with nc.allow_low_precision(reason="f32r == f32 bits"):
                        nc.vector.reciprocal(recB[:], ctxB[64:65, :])
                    rbB = pst.tile([64, TOK], F32, tag="st1")
                    nc.tensor.matmul(rbB[:], ones_row[:, 0:64], recB[:],
                                     start=True, stop=True)
                    rbBs = rows.tile([64, TOK], F32, tag="rbs")
                    nc.vector.tensor_copy(rbBs[:], rbB[:])
                    ctmp = rows.tile([64, TOK], F32R, tag="ctmp")
                    nc.vector.tensor_mul(ctmp[:], ctxB[0:64, :], rbBs[:])
                    nc.sync.dma_start(ctx_sb[64:128, hp, :], ctmp[:])

                # ---- W_o + residual ----
                ao = res.tile([P, DT, TOK], F32R, tag="res")
                for ob in [o for _ in range(rep_tail) for o in range(8)]:
                    wt = wts.tile([P, DT, 128], F32R, tag="wt")
                    nc.sync.dma_start(wt[:], wo_in[ob])
                    ps2 = pmm.tile([P, 2, TOK], F32, tag="mm2", name=f"ops{ob}")
                    ps = ps2[:, 0, :]
                    for ct in range(DT):
                        nc.tensor.matmul(ps, wt[:, ct, :], ctx_sb[:, ct, :],
                                         start=(ct == 0), stop=(ct == DT - 1))
                    nc.vector.tensor_add(ao[:, ob, :], ps, xt[:, ob, :].bitcast(F32))

                # ---- LN2 ----
                h2 = big.tile([P, DT, TOK], F32R, tag="h2")
                layernorm(ao, h2, 1)

                # ---- FFN ----
                zt = big.tile([P, DT, TOK], F32R, tag="act")
                for cb in [c for _ in range(rep_tail) for c in range(8)]:
                    wt = wts.tile([P, DT, 128], F32R, tag="wt")
                    nc.sync.dma_start(wt[:], w1_in[cb])
                    ps2 = pmm.tile([P, 2, TOK], F32, tag="mm2", name=f"zps{cb}")
                    ps = ps2[:, 0, :]
                    for dt in range(DT):
                        nc.tensor.matmul(ps, wt[:, dt, :], h2[:, dt, :],
                                         start=(dt == 0), stop=(dt == DT - 1))
                    nc.scalar.activation(zt[:, cb, :], ps, AF.Relu)
                f2 = res.tile([P, DT, TOK], F32R, tag="res")
                for ob in [o for _ in range(rep_tail) for o in range(8)]:
                    wt = wts.tile([P, DT, 128], F32R, tag="wt")
                    nc.sync.dma_start(wt[:], w2_in[ob])
                    ps2 = pmm.tile([P, 2, TOK], F32, tag="mm2", name=f"fps{ob}")
                    ps = ps2[:, 0, :]
                    for ct in range(DT):
                        nc.tensor.matmul(ps, wt[:, ct, :], zt[:, ct, :],
                                         start=(ct == 0), stop=(ct == DT - 1))
                    nc.vector.tensor_add(f2[:, ob, :], ps, h2[:, ob, :].bitcast(F32))

                # ---- LN3 + output (reuse layernorm, final ACT writes F32 tiles) ----
                scr = big.tile([P, DT, TOK], F32R, tag="scratch")
                nc.vector.tensor_mul(scr[:], f2[:].bitcast(F32), f2[:].bitcast(F32))
                st0 = pst.tile([1, TOK], F32, tag="st0")
                st1 = pst.tile([1, TOK], F32, tag="st1")
                for dt in range(DT):
                    nc.tensor.matmul(st0[:], ones_r[:], f2[:, dt, :],
                                     start=(dt == 0), stop=(dt == DT - 1))
                for dt in range(DT):
                    nc.tensor.matmul(st1[:], ones_r[:], scr[:, dt, :],
                                     start=(dt == 0), stop=(dt == DT - 1))
                mu = rows.tile([1, TOK], F32, tag="mu")
                var = rows.tile([1, TOK], F32, tag="var")
                msq = rows.tile([1, TOK], F32, tag="msq")
                sd = rows.tile([1, TOK], F32, tag="sd")
                row = rows.tile([1, 2 * TOK], F32R, tag="row")
                nc.vector.tensor_scalar_mul(mu[:], st0[:], 1.0 / D)
                nc.vector.tensor_scalar_mul(var[:], st1[:], 1.0 / D)
                nc.vector.tensor_mul(msq[:], mu[:], mu[:])
                nc.vector.tensor_tensor(var[:], var[:], msq[:], mybir.AluOpType.subtract)
                nc.scalar.activation(sd[:], var[:], AF.Sqrt, bias=eps_t[:], scale=1.0)
                with nc.allow_low_precision(reason="f32r == f32 bits"):
                    nc.vector.reciprocal(row[:, 0:TOK], sd[:])
                nc.vector.tensor_scalar_mul(msq[:], mu[:], -1.0)
                nc.vector.tensor_mul(row[:, TOK:], msq[:], row[:, 0:TOK])
                bcp = pmm.tile([P, 2, TOK], F32, tag="mm2")
                nc.tensor.matmul(bcp[:, 0, :], ones_row[:], row[:, 0:TOK],
                                 start=True, stop=True)
                nc.tensor.matmul(bcp[:, 1, :], ones_row[:], row[:, TOK:],
                                 start=True, stop=True)
                rs_b = bcp[:, 0, None, :].to_broadcast((P, DT, TOK))
                nb_b = bcp[:, 1, None, :].to_broadcast((P, DT, TOK))
                nc.vector.tensor_mul(scr[:], f2[:].bitcast(F32), rs_b)
                nc.vector.tensor_tensor(scr[:], scr[:].bitcast(F32), nb_b,
                                        mybir.AluOpType.add)
                for dt in range(DT):
                    ot = outs.tile([P, TOK], F32, tag="ot")
                    nc.scalar.activation(ot[:], scr[:, dt, :].bitcast(F32), AF.Identity,
                                         scale=gb[:, 2, 0, dt:dt + 1],
                                         bias=gb[:, 2, 1, dt:dt + 1])
                    nc.sync.dma_start(outT[:, dt, :], ot[:])

    nc.finalize()
    return nc


def _wcol(w):
    """[1024, 1024] (in, out) -> [8, 128, 8, 128] = [out_blk, p_in, in_tile, out_w]."""
    return np.ascontiguousarray(
        w.reshape(DT, P, 8, 128).transpose(2, 1, 0, 3)).astype(np.float32)


def prepare_in_maps(x, wq, wk, wv, wo, w1, w2,
                    ln1_g, ln1_b, ln2_g, ln2_b, ln3_g, ln3_b):
    x = np.asarray(x, np.float32)
    wq_f = np.asarray(wq, np.float32).transpose(1, 0, 2).reshape(D, D)
    wk_f = np.asarray(wk, np.float32).transpose(1, 0, 2).reshape(D, D)
    wv_f = np.asarray(wv, np.float32).transpose(1, 0, 2).reshape(D, D)
    wo_f = np.asarray(wo, np.float32)
    w1_f = np.asarray(w1, np.float32)
    w2_f = np.asarray(w2, np.float32)

    wq_a, wk_a, wo_a, w1_a, w2_a = map(_wcol, (wq_f, wk_f, wo_f, w1_f, w2_f))
    wv_a = np.ascontiguousarray(
        wv_f.reshape(DT, P, 2, 512).transpose(2, 0, 1, 3)).astype(np.float32)

    gb = np.zeros((P, 3, 2, DT), np.float32)
    for i, (g, b) in enumerate(((ln1_g, ln1_b), (ln2_g, ln2_b), (ln3_g, ln3_b))):
        gb[:, i, 0, :] = np.asarray(g, np.float32).reshape(DT, P).T
        gb[:, i, 1, :] = np.asarray(b, np.float32).reshape(DT, P).T

    x_flat = x.reshape(B * S, D)
    in_maps = []
    for c in range(NC):
        xs = x_flat[c * TOK:(c + 1) * TOK].T          # [D, TOK]
        xt = np.ascontiguousarray(xs.reshape(DT, P, TOK).transpose(1, 0, 2))
        in_maps.append({
            "xT": xt, "wq": wq_a, "wk": wk_a, "wv": wv_a,
            "wo": wo_a, "w1": w1_a, "w2": w2_a, "gb": gb,
        })

    return in_maps


def kernel(x, wq, wk, wv, wo, w1, w2,
           ln1_g, ln1_b, ln2_g, ln2_b, ln3_g, ln3_b):
    from concourse.bass_utils import run_bass_kernel_spmd

    in_maps = prepare_in_maps(x, wq, wk, wv, wo, w1, w2,
                              ln1_g, ln1_b, ln2_g, ln2_b, ln3_g, ln3_b)
    if "nc" not in _BUILT:
        _BUILT["nc"] = _build()
    last_exc = None
    for _attempt in range(3):
        try:
            res = run_bass_kernel_spmd(_BUILT["nc"], in_maps,
                                       core_ids=list(range(NC)), trace=TRACE)
            break
        except Exception as e:         # transient device wedge -> retry
            last_exc = e
            import time as _time
            _time.sleep(10)
    else:
        raise last_exc
    if TRACE and res.exec_time_ns is not None:
        _BUILT["exec_time_ns"] = res.exec_time_ns
        _BUILT["trace"] = res.instructions_and_trace

    parts = []
    for c in range(NC):
        arr = res.results[c]["outT"]                  # [P, DT, TOK]
        parts.append(arr.transpose(2, 1, 0).reshape(TOK, D))
    return np.concatenate(parts, axis=0).reshape(B, S, D).astype(np.float32)

